# revision 27
# baseline (speedup 1.0000x reference)
"""SMPL (nn_SMPL_33028298506333) Trainium2 Bass kernel.

Data-parallel over the batch N=2048 across 8 NeuronCores (256 bodies/core).
Each core computes, for its bodies, the full SMPL pipeline:

  host prep:  W^T [218, V*3]   (row 0 = v_template, 1..10 = shapedirs,
                                11..217 = posedirs)  -- feature-major layout
              w4 [128, V_pad]  (weights^T replicated at partition offsets
                                0/32/64/96 for tile_position row-tiling)
              packed J_regressor / joint_regressor / [v_template|shapedirs]

  device:
   n-row space (128 bodies / chunk, 2 chunks):
     rodrigues -> R, lrotmin; F = [1; beta; lrotmin] (PE-transposed to
     feature-rows); J = F[0:11]^T @ JD (JD = regressor x shape basis, computed
     on device once); kinematic chain + G_rel correction + global trans fold
     (strided DVE ops, batched per tree level); PE-transpose G -> G_T with
     rows (32*b + j).
   v-row space (54 tiles of 128 vertices):
     v_posed tile  = W^T_tile^T @ F            (2 K-chunks, fp32r)
     T slots (a,b) = w^T @ G_T slots           (K=24, 4x row-tiled)
     verts = sum_b T_ab * vp_b + T_a3          (DVE elementwise)
     joints += jreg_tile^T @ verts             (PSUM-accumulated over tiles)

Outputs are written v-major ([V_pad, 3, 256] per core) and re-laid-out on the
host to the reference's [N, V, 3] / [N, 19, 3].
"""

import sys

for _p in ("/opt/trn_rl_repo",):
    if _p not in sys.path:
        sys.path.insert(0, _p)

import numpy as np

import concourse.bass as bass
import concourse.mybir as mybir
from concourse import bacc
from concourse.tile import TileContext
from concourse.masks import make_identity

F32 = mybir.dt.float32
F32R = mybir.dt.float32r

V, NJ, NB, NP, NK = 6890, 24, 10, 207, 19
NKP = 20                        # NK padded even for fp32r matmul
N_FULL = 2048
NCORES = 8
NPC = N_FULL // NCORES          # 256 bodies per core
NCH = NPC // 128                # 2 n-chunks of 128
NF = 1 + NB + NP                # 218 features
NT = 54                         # vertex tiles of 128
V_PAD = NT * 128                # 6912
VC_PAD = V_PAD * 3              # 20736

PARENTS = [-1, 0, 0, 0, 1, 2, 3, 4, 5, 6, 7, 8, 9, 9, 9, 12, 13, 14, 16, 17,
           18, 19, 20, 21]
# (j0, cnt, p0, pstep): consecutive joint ranges whose parents are affine.
LEVELS = [(1, 3, 0, 0), (4, 3, 1, 1), (7, 3, 4, 1), (10, 3, 7, 1),
          (13, 2, 9, 0), (15, 3, 12, 1), (18, 2, 16, 1), (20, 2, 18, 1),
          (22, 2, 20, 1)]

USE_F32R = True
TINY = float(np.finfo(np.float32).tiny)
HALF_PI = float(np.pi / 2)

# skew entries: R[a,b] += sign * s*rh[k]
SKEW = [(0, 1, 2, -1), (0, 2, 1, +1), (1, 0, 2, +1),
        (1, 2, 0, -1), (2, 0, 1, -1), (2, 1, 0, +1)]

_CACHED_NC = None
MMDT = F32R if USE_F32R else F32


def _mmdt(ap):
    return ap


def _build_nc():
    nc = bacc.Bacc("TRN2", target_bir_lowering=False, debug=False)

    # ---- DRAM tensors ----
    wt = nc.dram_tensor("wt", [NF, VC_PAD], MMDT, kind="ExternalInput")
    w4 = nc.dram_tensor("w4", [128, V_PAD], MMDT, kind="ExternalInput")
    jregp = nc.dram_tensor("jregp", [128, NT * NKP], MMDT,
                           kind="ExternalInput")
    jd = nc.dram_tensor("jd", [11, 3 * NJ], F32, kind="ExternalInput")
    betaT = nc.dram_tensor("betaT", [NB + 1, NPC], MMDT,
                           kind="ExternalInput")
    theta = nc.dram_tensor("theta", [NPC, NJ * 3], F32, kind="ExternalInput")
    trans = nc.dram_tensor("trans", [NPC, 3], F32, kind="ExternalInput")
    verts_o = nc.dram_tensor("verts_o", [V_PAD, 3 * NPC], F32,
                             kind="ExternalOutput")
    joints_o = nc.dram_tensor("joints_o", [NK, 3 * NPC], F32,
                              kind="ExternalOutput")

    with TileContext(nc) as tc:
        import contextlib
        with contextlib.ExitStack() as ctx:
            _body(ctx, tc, nc, wt, w4, jregp, jd, betaT, theta,
                  trans, verts_o, joints_o)
    nc.compile()
    return nc


def _body(ctx, tc, nc, wt, w4, jregp, jd, betaT, theta, trans,
          verts_o, joints_o):
    mul = mybir.AluOpType.mult
    addop = mybir.AluOpType.add
    subop = mybir.AluOpType.subtract
    AX = mybir.AxisListType.X
    ACT_SIN = mybir.ActivationFunctionType.Sin
    ACT_SQRT = mybir.ActivationFunctionType.Sqrt

    # ------- persistent SBUF -------
    const = ctx.enter_context(tc.tile_pool(name="const", bufs=1))
    ident = const.tile([128, 128], F32)
    make_identity(nc, ident)
    halfpi = const.tile([128, 1], F32)
    nc.vector.memset(halfpi[:, :], HALF_PI)
    w4_sb = const.tile([128, V_PAD], MMDT)
    nc.sync.dma_start(w4_sb[:, :], w4[:, :])
    jreg_sb = const.tile([128, NT * NKP], MMDT)
    nc.sync.dma_start(jreg_sb[:, :], jregp[:, :])

    F_hi = const.tile([128, NPC], MMDT)     # feature rows 0..127
    F_lo = const.tile([128, NPC], MMDT)     # feature rows 128..217 on p 0..89
    G_T = const.tile([128, 3 * NPC], MMDT)  # rows 32*b+j ; col a*NPC + n
    jd_sb = const.tile([11, 3 * NJ], F32)   # JD, col c*24+j
    nc.sync.dma_start(jd_sb[:, :], jd[:, :])

    # ================= phase 1: per n-chunk (n-row space) ===================
    with tc.tile_pool(name="nrow", bufs=2) as nrow, \
         tc.tile_pool(name="nrow_tmp", bufs=3) as ntmp, \
         tc.tile_pool(name="ps1", bufs=2, space="PSUM") as ps1:
        for ch in range(NCH):
            ns = slice(ch * 128, (ch + 1) * 128)

            th = nrow.tile([128, NJ * 3], F32, tag="theta")
            nc.sync.dma_start(th[:, :], theta[ns, :])
            tr = nrow.tile([128, 3], F32, tag="trans")
            nc.sync.dma_start(tr[:, :], trans[ns, :])
            nc.sync.dma_start(F_hi[0:11, ns], betaT[:, ns])

            th3 = th[:, :].rearrange("p (j c) -> p j c", c=3)

            # ---- rodrigues ----
            sq = ntmp.tile([128, NJ * 3], F32, tag="sq")
            nc.vector.tensor_mul(sq[:, :], th[:, :], th[:, :])
            sq3 = sq[:, :].rearrange("p (j c) -> p j c", c=3)
            nsq = ntmp.tile([128, NJ], F32, tag="nsq")
            nc.vector.tensor_add(nsq[:, :], sq3[:, :, 0], sq3[:, :, 1])
            nc.vector.tensor_add(nsq[:, :], nsq[:, :], sq3[:, :, 2])
            norm = ntmp.tile([128, NJ], F32, tag="norm")
            nc.scalar.activation(norm[:, :], nsq[:, :], ACT_SQRT)
            nc.vector.tensor_scalar_max(norm[:, :], norm[:, :], TINY)
            inv = ntmp.tile([128, NJ], F32, tag="inv")
            nc.vector.reciprocal(inv[:, :], norm[:, :])
            rh = ntmp.tile([128, NJ * 3], F32, tag="rh")
            rh3 = rh[:, :].rearrange("p (j c) -> p j c", c=3)
            inv_b = inv[:, :].unsqueeze(2).broadcast_to([128, NJ, 3])
            nc.vector.tensor_mul(rh3, th3, inv_b)
            cosx = ntmp.tile([128, NJ], F32, tag="cosx")
            nc.scalar.activation(cosx[:, :], norm[:, :], ACT_SIN,
                                 bias=halfpi[:, :])
            sinx = ntmp.tile([128, NJ], F32, tag="sinx")
            nc.scalar.activation(sinx[:, :], norm[:, :], ACT_SIN)
            omc = ntmp.tile([128, NJ], F32, tag="omc")
            nc.vector.tensor_scalar(omc[:, :], cosx[:, :], -1.0, 1.0, mul,
                                    addop)

            R = ntmp.tile([128, NJ * 9], F32, tag="R")
            R4 = R[:, :].rearrange("p (j a b) -> p j a b", a=3, b=3)
            rh_a = rh3.unsqueeze(3).broadcast_to([128, NJ, 3, 3])
            rh_b = rh3.unsqueeze(2).broadcast_to([128, NJ, 3, 3])
            nc.vector.tensor_mul(R4, rh_a, rh_b)
            omc_b = omc[:, :].unsqueeze(2).unsqueeze(3).broadcast_to(
                [128, NJ, 3, 3])
            nc.vector.tensor_mul(R4, R4, omc_b)
            for d in range(3):
                nc.vector.tensor_add(R4[:, :, d, d], R4[:, :, d, d],
                                     cosx[:, :])
            sr = ntmp.tile([128, NJ * 3], F32, tag="sr")
            sr3 = sr[:, :].rearrange("p (j c) -> p j c", c=3)
            sinx_b = sinx[:, :].unsqueeze(2).broadcast_to([128, NJ, 3])
            nc.vector.tensor_mul(sr3, rh3, sinx_b)
            for (a, b, k, sg) in SKEW:
                nc.vector.tensor_tensor(
                    R4[:, :, a, b], R4[:, :, a, b], sr3[:, :, k],
                    addop if sg > 0 else subop)

            # ---- lrotmin -> (transposed) F rows 11..217 ----
            lrot = ntmp.tile([128, 256], F32, tag="lrot")
            nc.vector.memset(lrot[:, NP:256], 0.0)
            nc.vector.tensor_copy(lrot[:, 0:NP], R[:, 9:9 + NP])
            l3 = lrot[:, 0:NP].rearrange("p (j e) -> p j e", e=9)
            for d in range(3):
                nc.vector.tensor_scalar(l3[:, :, 4 * d], l3[:, :, 4 * d],
                                        1.0, None, subop)
            tp1 = ps1.tile([128, 128], F32, tag="tp")
            nc.tensor.transpose(tp1[:, :], lrot[:, 0:128], ident[:, :])
            stage1 = ntmp.tile([128, 128], MMDT, tag="stage1")
            nc.scalar.copy(stage1[:, :], tp1[:, :])
            nc.sync.dma_start(F_hi[11:128, ns], stage1[0:117, :])
            nc.sync.dma_start(F_lo[0:11, ns], stage1[117:128, :])
            tp2 = ps1.tile([128, 128], F32, tag="tp")
            nc.tensor.transpose(tp2[:, :], lrot[:, 128:256], ident[:, :])
            stage2 = ntmp.tile([128, 128], MMDT, tag="stage2")
            nc.scalar.copy(stage2[0:96, :], tp2[0:96, :])
            nc.sync.dma_start(F_lo[11:90, ns], stage2[0:79, :])

            # ---- J regression (uses only feature rows 0..10) ----
            j_ps = ps1.tile([128, NJ * 3], F32, tag="jps")
            jd_v = jd_sb[0:11, :].rearrange("p (c j) -> p j c", c=3)
            nc.tensor.matmul(j_ps[:, :], F_hi[0:11, ns].bitcast(F32), jd_v,
                             start=True, stop=True)
            J = nrow.tile([128, NJ * 3], F32, tag="J")
            nc.scalar.copy(J[:, :], j_ps[:, :])
            J3 = J[:, :].rearrange("p (j c) -> p j c", c=3)

            # ---- A matrices (3x4 per joint), col (a*4+b)*32 + j ----
            A = nrow.tile([128, 384], F32, tag="A")
            A4 = A[:, :].rearrange("p (a b j) -> p a b j", a=3, b=4, j=32)
            nc.vector.memset(A[:, :], 0.0)
            # rotation entries
            nc.vector.tensor_copy(A4[:, :, 0:3, 0:NJ],
                                  R4.transpose([0, 2, 3, 1]))
            # translation col: root then levels (t_j = J_j - J_parent)
            nc.vector.tensor_copy(A4[:, :, 3, 0:1],
                                  J3[:, 0:1, :].transpose([0, 2, 1]))
            for (j0, cnt, p0, pstep) in LEVELS:
                dst = A4[:, :, 3, j0:j0 + cnt]
                jj = J3[:, j0:j0 + cnt, :].transpose([0, 2, 1])
                if pstep == 0:
                    jp = J3[:, p0:p0 + 1, :].transpose([0, 2, 1]) \
                        .broadcast_to([128, 3, cnt])
                else:
                    jp = J3[:, p0:p0 + cnt, :].transpose([0, 2, 1])
                nc.vector.tensor_sub(dst, jj, jp)

            # ---- kinematic chain ----
            G = nrow.tile([128, 384], F32, tag="G")
            G4 = G[:, :].rearrange("p (a b j) -> p a b j", a=3, b=4, j=32)
            # root
            nc.vector.tensor_copy(G4[:, :, :, 0:1], A4[:, :, :, 0:1])
            for (j0, cnt, p0, pstep) in LEVELS:
                tmp = ntmp.tile([128, cnt * 36], F32, tag="chn%d" % cnt)
                t5 = tmp[:, :].rearrange("p (j a b k) -> p j a b k",
                                         j=cnt, a=3, b=4, k=3)
                # ISA allows <=3 free dims per op: split over a.
                for a in range(3):
                    if pstep == 0:
                        gp = G4[:, a, 0:3, p0:p0 + 1].transpose([0, 2, 1]) \
                            .unsqueeze(2).broadcast_to([128, cnt, 4, 3])
                    else:
                        gp = G4[:, a, 0:3, p0:p0 + cnt].transpose([0, 2, 1]) \
                            .unsqueeze(2).broadcast_to([128, cnt, 4, 3])
                    aa = A4[:, 0:3, :, j0:j0 + cnt].transpose([0, 3, 2, 1])
                    nc.vector.tensor_mul(t5[:, :, a, :, :], gp, aa)
                    gout = G4[:, a, :, j0:j0 + cnt].transpose([0, 2, 1])
                    nc.vector.tensor_reduce(gout, t5[:, :, a, :, :], AX,
                                            mybir.AluOpType.add)
                # + Gp[a,3] into b=3 column
                if pstep == 0:
                    gpt = G4[:, :, 3, p0:p0 + 1].broadcast_to([128, 3, cnt])
                else:
                    gpt = G4[:, :, 3, p0:p0 + cnt]
                nc.vector.tensor_add(G4[:, :, 3, j0:j0 + cnt],
                                     G4[:, :, 3, j0:j0 + cnt], gpt)

            # ---- G_rel: subtract G_rot @ J from translation col ----
            ctmp = ntmp.tile([128, NJ * 9], F32, tag="ctmp")
            c4 = ctmp[:, :].rearrange("p (j a b) -> p j a b", a=3, b=3)
            gj = G4[:, :, 0:3, 0:NJ].transpose([0, 3, 1, 2])
            jb = J3.unsqueeze(2).broadcast_to([128, NJ, 3, 3])
            nc.vector.tensor_mul(c4, gj, jb)
            csum = ntmp.tile([128, NJ * 3], F32, tag="csum")
            cs3 = csum[:, :].rearrange("p (j a) -> p j a", a=3)
            nc.vector.tensor_reduce(cs3, c4, AX, mybir.AluOpType.add)
            nc.vector.tensor_sub(G4[:, :, 3, 0:NJ], G4[:, :, 3, 0:NJ],
                                 cs3.transpose([0, 2, 1]))
            # fold global translation (weights rows sum to 1)
            tr_b = tr[:, :].unsqueeze(2).broadcast_to([128, 3, NJ])
            nc.vector.tensor_add(G4[:, :, 3, 0:NJ], G4[:, :, 3, 0:NJ], tr_b)

            # ---- transpose G into G_T (rows 32b+j), col a*NPC + ch*128+n --
            for a in range(3):
                gt_ps = ps1.tile([128, 128], F32, tag="gt")
                nc.tensor.transpose(gt_ps[:, :], G[:, a * 128:(a + 1) * 128],
                                    ident[:, :])
                nc.scalar.copy(G_T[:, a * NPC + ch * 128:
                                   a * NPC + ch * 128 + 128], gt_ps[:, :])

    # ================= phase 2: vertex tiles ================================
    wpool = ctx.enter_context(tc.tile_pool(name="wstream", bufs=4))
    mpool = ctx.enter_context(tc.tile_pool(name="mtiles", bufs=6))
    vout = ctx.enter_context(tc.tile_pool(name="vout", bufs=5))
    vppool = ctx.enter_context(tc.tile_pool(name="vpsb", bufs=3))
    ps_vp_pool = ctx.enter_context(
        tc.tile_pool(name="psvp", bufs=1, space="PSUM"))
    ps_t_pool = ctx.enter_context(
        tc.tile_pool(name="pst", bufs=2, space="PSUM"))
    ps_jt_pool = ctx.enter_context(
        tc.tile_pool(name="psjt", bufs=1, space="PSUM"))

    jt_ps = [ps_jt_pool.tile([NKP, 384], F32, tag="jt%d" % h,
                             name="jt_ps%d" % h)
             for h in range(2)]

    for vt in range(NT):
        cs = slice(vt * 384, (vt + 1) * 384)
        wt_hi = wpool.tile([128, 384], MMDT, tag="whi")
        nc.sync.dma_start(wt_hi[:, :], wt[0:128, cs])
        wt_lo = wpool.tile([128, 384], MMDT, tag="wlo")
        nc.sync.dma_start(wt_lo[0:90, :], wt[128:NF, cs])

        # ---- v_posed tile:  [128v, (c,n)] ----
        ps_vp = ps_vp_pool.tile([128, 3 * NPC], F32, tag="vp", name="ps_vp")
        whi3 = wt_hi[:, :].rearrange("p (v c) -> p c v", c=3)
        wlo3 = wt_lo[0:90, :].rearrange("p (v c) -> p c v", c=3)
        for c in range(3):
            nc.tensor.matmul(ps_vp[:, c * NPC:(c + 1) * NPC],
                             _mmdt(whi3[:, c, :]), _mmdt(F_hi[:, :]),
                             start=True, stop=False)
            nc.tensor.matmul(ps_vp[:, c * NPC:(c + 1) * NPC],
                             _mmdt(wlo3[:, c, :]), _mmdt(F_lo[0:90, :]),
                             start=False, stop=True)
        vp_sb = vppool.tile([128, 3 * NPC], F32, tag="vpsb")
        for c in range(3):
            nc.scalar.copy(vp_sb[:, c * NPC:(c + 1) * NPC],
                           ps_vp[:, c * NPC:(c + 1) * NPC])

        # ---- T slots + stage B ----
        m = []
        ps_t3 = None
        for b in range(4):
            ps_t = ps_t_pool.tile([128, 3 * NPC], F32, tag="pst", name="ps_t")
            # two mms (N=512 bank0, N=256 bank1) instead of three N=256:
            # fewer LDWEIGHTS and better fp32r streaming rate.
            for (o, w) in ((0, 512), (512, 256)):
                nc.tensor.matmul(
                    ps_t[:, o:o + w],
                    _mmdt(w4_sb[32 * b:32 * b + NJ,
                                vt * 128:(vt + 1) * 128]),
                    _mmdt(G_T[32 * b:32 * b + NJ, o:o + w]),
                    start=True, stop=True, tile_position=(32 * b, 0))
            if b < 3:
                mb = mpool.tile([128, 3 * NPC], F32, tag="m%d" % b)
                m3 = mb[:, :].rearrange("p (a n) -> p a n", n=NPC)
                t3 = ps_t[:, :].rearrange("p (a n) -> p a n", n=NPC)
                vp_b = vp_sb[:, b * NPC:(b + 1) * NPC].unsqueeze(1) \
                    .broadcast_to([128, 3, NPC])
                nc.vector.tensor_mul(m3, t3, vp_b)
                m.append(mb)
            else:
                ps_t3 = ps_t

        # adds on GpSimd: fp32 tensor_tensor on DVE is 1x/single-port, so
        # GpSimd runs fully in parallel (no shared-port contention).
        s1 = mpool.tile([128, 3 * NPC], F32, tag="s1")
        nc.gpsimd.tensor_tensor(s1[:, :], m[0][:, :], m[1][:, :],
                                mybir.AluOpType.add)
        s2 = mpool.tile([128, 3 * NPC], F32, tag="s2")
        nc.gpsimd.tensor_tensor(s2[:, :], s1[:, :], m[2][:, :],
                                mybir.AluOpType.add)
        verts_sb = vout.tile([128, 3 * NPC], F32, tag="verts")
        nc.vector.tensor_add(verts_sb[:, :], s2[:, :], ps_t3[:, :])

        # ---- joints accumulation (rounded copy keeps verts_sb full fp32) --
        if USE_F32R:
            verts_r = vout.tile([128, 3 * NPC], MMDT, tag="verts_r")
            nc.scalar.copy(verts_r[:, :], verts_sb[:, :])
            jrhs = verts_r
        else:
            jrhs = verts_sb
        for h in range(2):
            nc.tensor.matmul(jt_ps[h][0:NKP, :],
                             _mmdt(jreg_sb[:, vt * NKP:(vt + 1) * NKP]),
                             _mmdt(jrhs[:, h * 384:(h + 1) * 384]),
                             start=(vt == 0), stop=(vt == NT - 1))

        nc.sync.dma_start(verts_o[vt * 128:(vt + 1) * 128, :], verts_sb[:, :])

    joints_sb = vout.tile([NKP, 3 * NPC], F32, tag="joints")
    for h in range(2):
        nc.scalar.copy(joints_sb[0:NKP, h * 384:(h + 1) * 384],
                       jt_ps[h][0:NKP, :])
    nc.sync.dma_start(joints_o[:, :], joints_sb[0:NK, :])


# ======================= host side ========================================

def _prep_shared(shapedirs, v_template, J_regressor, posedirs, weights,
                 joint_regressor):
    wt = np.zeros((NF, VC_PAD), np.float32)
    wt[0, :V * 3] = v_template.ravel()
    wt[1:1 + NB, :V * 3] = shapedirs.transpose(2, 0, 1).reshape(NB, -1)
    wt[1 + NB:, :V * 3] = posedirs.transpose(2, 0, 1).reshape(NP, -1)

    w4 = np.zeros((128, V_PAD), np.float32)
    for u in range(4):
        w4[32 * u:32 * u + NJ, :V] = weights.T

    jr = np.zeros((V_PAD, NKP), np.float32)
    jr[:V, :NK] = joint_regressor
    jregp = np.ascontiguousarray(
        jr.reshape(NT, 128, NKP).transpose(1, 0, 2).reshape(128, NT * NKP))

    # JD[f, c*24+j] = sum_v [v_template|shapedirs][v,c,f] * J_regressor[j,v]
    jd = np.zeros((11, 3 * NJ), np.float32)
    for c in range(3):
        W_c = np.concatenate([v_template[:, c:c + 1], shapedirs[:, c, :]],
                             axis=1)                      # [V, 11]
        jd[:, c * NJ:(c + 1) * NJ] = (W_c.T @ J_regressor.T).astype(
            np.float32)

    return wt, w4, jregp, jd


def make_in_maps(inputs):
    """inputs: dict of full-size numpy arrays keyed as in setup_inputs()."""
    beta = np.asarray(inputs["beta"], np.float32)
    theta = np.asarray(inputs["theta"], np.float32)
    trans = np.asarray(inputs["trans"], np.float32)
    wt, w4, jregp, jd = _prep_shared(
        np.asarray(inputs["shapedirs"], np.float32),
        np.asarray(inputs["v_template"], np.float32),
        np.asarray(inputs["J_regressor"], np.float32),
        np.asarray(inputs["posedirs"], np.float32),
        np.asarray(inputs["weights"], np.float32),
        np.asarray(inputs["joint_regressor"], np.float32))

    in_maps = []
    for r in range(NCORES):
        ns = slice(r * NPC, (r + 1) * NPC)
        in_maps.append({
            "wt": wt, "w4": w4, "jregp": jregp, "jd": jd,
            "betaT": np.ascontiguousarray(
                np.concatenate([np.ones((1, NPC), np.float32),
                                beta[ns].T], axis=0)),
            "theta": np.ascontiguousarray(theta[ns].reshape(NPC, NJ * 3)),
            "trans": np.ascontiguousarray(trans[ns]),
        })
    return in_maps


def unshard(results):
    verts = np.empty((N_FULL, V, 3), np.float32)
    joints = np.empty((N_FULL, NK, 3), np.float32)
    for r, res in enumerate(results):
        ns = slice(r * NPC, (r + 1) * NPC)
        vo = res["verts_o"].reshape(V_PAD, 3, NPC)[:V]
        verts[ns] = vo.transpose(2, 0, 1)
        jo = res["joints_o"].reshape(NK, 3, NPC)
        joints[ns] = jo.transpose(2, 0, 1)
    return verts, joints


def get_nc():
    global _CACHED_NC
    if _CACHED_NC is None:
        _CACHED_NC = _build_nc()
    return _CACHED_NC


def kernel(**inputs):
    from concourse.bass_utils import run_bass_kernel_spmd
    nc = get_nc()
    in_maps = make_in_maps(inputs)
    res = run_bass_kernel_spmd(nc, in_maps, core_ids=list(range(NCORES)))
    return unshard(res.results)


# revision 28
# speedup vs baseline: 1.1360x; 1.1360x over previous
"""SMPL (nn_SMPL_33028298506333) Trainium2 Bass kernel.

Data-parallel over the batch N=2048 across 8 NeuronCores (256 bodies/core).
Each core computes, for its bodies, the full SMPL pipeline:

  host prep:  W^T [218, V*3]   (row 0 = v_template, 1..10 = shapedirs,
                                11..217 = posedirs)  -- feature-major layout
              w4 [128, V_pad]  (weights^T replicated at partition offsets
                                0/32/64/96 for tile_position row-tiling)
              packed J_regressor / joint_regressor / [v_template|shapedirs]

  device:
   n-row space (128 bodies / chunk, 2 chunks):
     rodrigues -> R, lrotmin; F = [1; beta; lrotmin] (PE-transposed to
     feature-rows); J = F[0:11]^T @ JD (JD = regressor x shape basis, computed
     on device once); kinematic chain + G_rel correction + global trans fold
     (strided DVE ops, batched per tree level); PE-transpose G -> G_T with
     rows (32*b + j).
   v-row space (54 tiles of 128 vertices):
     v_posed tile  = W^T_tile^T @ F            (2 K-chunks, fp32r)
     T slots (a,b) = w^T @ G_T slots           (K=24, 4x row-tiled)
     verts = sum_b T_ab * vp_b + T_a3          (DVE elementwise)
     joints += jreg_tile^T @ verts             (PSUM-accumulated over tiles)

Outputs are written v-major ([V_pad, 3, 256] per core) and re-laid-out on the
host to the reference's [N, V, 3] / [N, 19, 3].
"""

import sys

for _p in ("/opt/trn_rl_repo",):
    if _p not in sys.path:
        sys.path.insert(0, _p)

import numpy as np

import concourse.bass as bass
import concourse.mybir as mybir
from concourse import bacc
from concourse.tile import TileContext
from concourse.masks import make_identity

F32 = mybir.dt.float32
F32R = mybir.dt.float32r

V, NJ, NB, NP, NK = 6890, 24, 10, 207, 19
NKP = 20                        # NK padded even for fp32r matmul
N_FULL = 2048
NCORES = 8
NPC = N_FULL // NCORES          # 256 bodies per core
NCH = NPC // 128                # 2 n-chunks of 128
NF = 1 + NB + NP                # 218 features
NT = 54                         # vertex tiles of 128
V_PAD = NT * 128                # 6912
VC_PAD = V_PAD * 3              # 20736

PARENTS = [-1, 0, 0, 0, 1, 2, 3, 4, 5, 6, 7, 8, 9, 9, 9, 12, 13, 14, 16, 17,
           18, 19, 20, 21]
# (j0, cnt, p0, pstep): consecutive joint ranges whose parents are affine.
LEVELS = [(1, 3, 0, 0), (4, 3, 1, 1), (7, 3, 4, 1), (10, 3, 7, 1),
          (13, 2, 9, 0), (15, 3, 12, 1), (18, 2, 16, 1), (20, 2, 18, 1),
          (22, 2, 20, 1)]

USE_F32R = True
TINY = float(np.finfo(np.float32).tiny)
HALF_PI = float(np.pi / 2)

# skew entries: R[a,b] += sign * s*rh[k]
SKEW = [(0, 1, 2, -1), (0, 2, 1, +1), (1, 0, 2, +1),
        (1, 2, 0, -1), (2, 0, 1, -1), (2, 1, 0, +1)]

_CACHED_NC = None
MMDT = F32R if USE_F32R else F32


def _mmdt(ap):
    return ap


def _build_nc():
    nc = bacc.Bacc("TRN2", target_bir_lowering=False, debug=False)

    # ---- DRAM tensors ----
    wt = nc.dram_tensor("wt", [NF, VC_PAD], MMDT, kind="ExternalInput")
    w4 = nc.dram_tensor("w4", [128, V_PAD], MMDT, kind="ExternalInput")
    jregp = nc.dram_tensor("jregp", [128, NT * NKP], MMDT,
                           kind="ExternalInput")
    jd = nc.dram_tensor("jd", [11, 3 * NJ], F32, kind="ExternalInput")
    betaT = nc.dram_tensor("betaT", [NB + 1, NPC], MMDT,
                           kind="ExternalInput")
    theta = nc.dram_tensor("theta", [NPC, NJ * 3], F32, kind="ExternalInput")
    trans = nc.dram_tensor("trans", [NPC, 3], F32, kind="ExternalInput")
    verts_o = nc.dram_tensor("verts_o", [V_PAD, 3 * NPC], F32,
                             kind="ExternalOutput")
    joints_o = nc.dram_tensor("joints_o", [NK, 3 * NPC], F32,
                              kind="ExternalOutput")

    with TileContext(nc) as tc:
        import contextlib
        with contextlib.ExitStack() as ctx:
            _body(ctx, tc, nc, wt, w4, jregp, jd, betaT, theta,
                  trans, verts_o, joints_o)
    nc.compile()
    return nc


def _body(ctx, tc, nc, wt, w4, jregp, jd, betaT, theta, trans,
          verts_o, joints_o):
    mul = mybir.AluOpType.mult
    addop = mybir.AluOpType.add
    subop = mybir.AluOpType.subtract
    AX = mybir.AxisListType.X
    ACT_SIN = mybir.ActivationFunctionType.Sin
    ACT_SQRT = mybir.ActivationFunctionType.Sqrt

    # ------- persistent SBUF -------
    const = ctx.enter_context(tc.tile_pool(name="const", bufs=1))
    ident = const.tile([128, 128], F32)
    make_identity(nc, ident)
    halfpi = const.tile([128, 1], F32)
    nc.vector.memset(halfpi[:, :], HALF_PI)
    w4_sb = const.tile([128, V_PAD], MMDT)
    nc.sync.dma_start(w4_sb[:, :], w4[:, :])
    jreg_sb = const.tile([128, NT * NKP], MMDT)
    nc.sync.dma_start(jreg_sb[:, :], jregp[:, :])

    F_hi = const.tile([128, NPC], MMDT)     # feature rows 0..127
    F_lo = const.tile([128, NPC], MMDT)     # feature rows 128..217 on p 0..89
    G_T = const.tile([128, 3 * NPC], MMDT)  # rows 32*b+j ; col a*NPC + n
    jd_sb = const.tile([11, 3 * NJ], F32)   # JD, col c*24+j
    nc.sync.dma_start(jd_sb[:, :], jd[:, :])

    # ================= phase 1: per n-chunk (n-row space) ===================
    with tc.tile_pool(name="nrow", bufs=1) as nrow, \
         tc.tile_pool(name="nrow_tmp", bufs=2) as ntmp, \
         tc.tile_pool(name="ps1", bufs=2, space="PSUM") as ps1:
        for ch in range(NCH):
            ns = slice(ch * 128, (ch + 1) * 128)

            th = nrow.tile([128, NJ * 3], F32, tag="theta")
            nc.sync.dma_start(th[:, :], theta[ns, :])
            tr = nrow.tile([128, 3], F32, tag="trans")
            nc.sync.dma_start(tr[:, :], trans[ns, :])
            nc.sync.dma_start(F_hi[0:11, ns], betaT[:, ns])

            th3 = th[:, :].rearrange("p (j c) -> p j c", c=3)

            # ---- rodrigues ----
            sq = ntmp.tile([128, NJ * 3], F32, tag="sq")
            nc.vector.tensor_mul(sq[:, :], th[:, :], th[:, :])
            sq3 = sq[:, :].rearrange("p (j c) -> p j c", c=3)
            nsq = ntmp.tile([128, NJ], F32, tag="nsq")
            nc.vector.tensor_add(nsq[:, :], sq3[:, :, 0], sq3[:, :, 1])
            nc.vector.tensor_add(nsq[:, :], nsq[:, :], sq3[:, :, 2])
            norm = ntmp.tile([128, NJ], F32, tag="norm")
            nc.scalar.activation(norm[:, :], nsq[:, :], ACT_SQRT)
            nc.vector.tensor_scalar_max(norm[:, :], norm[:, :], TINY)
            inv = ntmp.tile([128, NJ], F32, tag="inv")
            nc.vector.reciprocal(inv[:, :], norm[:, :])
            rh = ntmp.tile([128, NJ * 3], F32, tag="rh")
            rh3 = rh[:, :].rearrange("p (j c) -> p j c", c=3)
            inv_b = inv[:, :].unsqueeze(2).broadcast_to([128, NJ, 3])
            nc.vector.tensor_mul(rh3, th3, inv_b)
            cosx = ntmp.tile([128, NJ], F32, tag="cosx")
            nc.scalar.activation(cosx[:, :], norm[:, :], ACT_SIN,
                                 bias=halfpi[:, :])
            sinx = ntmp.tile([128, NJ], F32, tag="sinx")
            nc.scalar.activation(sinx[:, :], norm[:, :], ACT_SIN)
            omc = ntmp.tile([128, NJ], F32, tag="omc")
            nc.vector.tensor_scalar(omc[:, :], cosx[:, :], -1.0, 1.0, mul,
                                    addop)

            R = ntmp.tile([128, NJ * 9], F32, tag="R")
            R4 = R[:, :].rearrange("p (j a b) -> p j a b", a=3, b=3)
            rh_a = rh3.unsqueeze(3).broadcast_to([128, NJ, 3, 3])
            rh_b = rh3.unsqueeze(2).broadcast_to([128, NJ, 3, 3])
            nc.vector.tensor_mul(R4, rh_a, rh_b)
            omc_b = omc[:, :].unsqueeze(2).unsqueeze(3).broadcast_to(
                [128, NJ, 3, 3])
            nc.vector.tensor_mul(R4, R4, omc_b)
            for d in range(3):
                nc.vector.tensor_add(R4[:, :, d, d], R4[:, :, d, d],
                                     cosx[:, :])
            sr = ntmp.tile([128, NJ * 3], F32, tag="sr")
            sr3 = sr[:, :].rearrange("p (j c) -> p j c", c=3)
            sinx_b = sinx[:, :].unsqueeze(2).broadcast_to([128, NJ, 3])
            nc.vector.tensor_mul(sr3, rh3, sinx_b)
            for (a, b, k, sg) in SKEW:
                nc.vector.tensor_tensor(
                    R4[:, :, a, b], R4[:, :, a, b], sr3[:, :, k],
                    addop if sg > 0 else subop)

            # ---- lrotmin -> (transposed) F rows 11..217 ----
            lrot = ntmp.tile([128, 256], F32, tag="lrot")
            nc.vector.memset(lrot[:, NP:256], 0.0)
            nc.vector.tensor_copy(lrot[:, 0:NP], R[:, 9:9 + NP])
            l3 = lrot[:, 0:NP].rearrange("p (j e) -> p j e", e=9)
            for d in range(3):
                nc.vector.tensor_scalar(l3[:, :, 4 * d], l3[:, :, 4 * d],
                                        1.0, None, subop)
            tp1 = ps1.tile([128, 128], F32, tag="tp")
            nc.tensor.transpose(tp1[:, :], lrot[:, 0:128], ident[:, :])
            stage1 = ntmp.tile([128, 128], MMDT, tag="stage1")
            nc.scalar.copy(stage1[:, :], tp1[:, :])
            nc.sync.dma_start(F_hi[11:128, ns], stage1[0:117, :])
            nc.sync.dma_start(F_lo[0:11, ns], stage1[117:128, :])
            tp2 = ps1.tile([128, 128], F32, tag="tp")
            nc.tensor.transpose(tp2[:, :], lrot[:, 128:256], ident[:, :])
            stage2 = ntmp.tile([128, 128], MMDT, tag="stage2")
            nc.scalar.copy(stage2[0:96, :], tp2[0:96, :])
            nc.sync.dma_start(F_lo[11:90, ns], stage2[0:79, :])

            # ---- J regression (uses only feature rows 0..10) ----
            j_ps = ps1.tile([128, NJ * 3], F32, tag="jps")
            jd_v = jd_sb[0:11, :].rearrange("p (c j) -> p j c", c=3)
            nc.tensor.matmul(j_ps[:, :], F_hi[0:11, ns].bitcast(F32), jd_v,
                             start=True, stop=True)
            J = nrow.tile([128, NJ * 3], F32, tag="J")
            nc.scalar.copy(J[:, :], j_ps[:, :])
            J3 = J[:, :].rearrange("p (j c) -> p j c", c=3)

            # ---- A matrices (3x4 per joint), col (a*4+b)*32 + j ----
            A = nrow.tile([128, 384], F32, tag="A")
            A4 = A[:, :].rearrange("p (a b j) -> p a b j", a=3, b=4, j=32)
            nc.vector.memset(A[:, :], 0.0)
            # rotation entries
            nc.vector.tensor_copy(A4[:, :, 0:3, 0:NJ],
                                  R4.transpose([0, 2, 3, 1]))
            # translation col: root then levels (t_j = J_j - J_parent)
            nc.vector.tensor_copy(A4[:, :, 3, 0:1],
                                  J3[:, 0:1, :].transpose([0, 2, 1]))
            for (j0, cnt, p0, pstep) in LEVELS:
                dst = A4[:, :, 3, j0:j0 + cnt]
                jj = J3[:, j0:j0 + cnt, :].transpose([0, 2, 1])
                if pstep == 0:
                    jp = J3[:, p0:p0 + 1, :].transpose([0, 2, 1]) \
                        .broadcast_to([128, 3, cnt])
                else:
                    jp = J3[:, p0:p0 + cnt, :].transpose([0, 2, 1])
                nc.vector.tensor_sub(dst, jj, jp)

            # ---- kinematic chain ----
            G = nrow.tile([128, 384], F32, tag="G")
            G4 = G[:, :].rearrange("p (a b j) -> p a b j", a=3, b=4, j=32)
            # root
            nc.vector.tensor_copy(G4[:, :, :, 0:1], A4[:, :, :, 0:1])
            for (j0, cnt, p0, pstep) in LEVELS:
                tmp = ntmp.tile([128, cnt * 36], F32, tag="chn%d" % cnt)
                t5 = tmp[:, :].rearrange("p (j a b k) -> p j a b k",
                                         j=cnt, a=3, b=4, k=3)
                # ISA allows <=3 free dims per op: split over a.
                for a in range(3):
                    if pstep == 0:
                        gp = G4[:, a, 0:3, p0:p0 + 1].transpose([0, 2, 1]) \
                            .unsqueeze(2).broadcast_to([128, cnt, 4, 3])
                    else:
                        gp = G4[:, a, 0:3, p0:p0 + cnt].transpose([0, 2, 1]) \
                            .unsqueeze(2).broadcast_to([128, cnt, 4, 3])
                    aa = A4[:, 0:3, :, j0:j0 + cnt].transpose([0, 3, 2, 1])
                    nc.vector.tensor_mul(t5[:, :, a, :, :], gp, aa)
                    gout = G4[:, a, :, j0:j0 + cnt].transpose([0, 2, 1])
                    nc.vector.tensor_reduce(gout, t5[:, :, a, :, :], AX,
                                            mybir.AluOpType.add)
                # + Gp[a,3] into b=3 column
                if pstep == 0:
                    gpt = G4[:, :, 3, p0:p0 + 1].broadcast_to([128, 3, cnt])
                else:
                    gpt = G4[:, :, 3, p0:p0 + cnt]
                nc.vector.tensor_add(G4[:, :, 3, j0:j0 + cnt],
                                     G4[:, :, 3, j0:j0 + cnt], gpt)

            # ---- G_rel: subtract G_rot @ J from translation col ----
            ctmp = ntmp.tile([128, NJ * 9], F32, tag="ctmp")
            c4 = ctmp[:, :].rearrange("p (j a b) -> p j a b", a=3, b=3)
            gj = G4[:, :, 0:3, 0:NJ].transpose([0, 3, 1, 2])
            jb = J3.unsqueeze(2).broadcast_to([128, NJ, 3, 3])
            nc.vector.tensor_mul(c4, gj, jb)
            csum = ntmp.tile([128, NJ * 3], F32, tag="csum")
            cs3 = csum[:, :].rearrange("p (j a) -> p j a", a=3)
            nc.vector.tensor_reduce(cs3, c4, AX, mybir.AluOpType.add)
            nc.vector.tensor_sub(G4[:, :, 3, 0:NJ], G4[:, :, 3, 0:NJ],
                                 cs3.transpose([0, 2, 1]))
            # fold global translation (weights rows sum to 1)
            tr_b = tr[:, :].unsqueeze(2).broadcast_to([128, 3, NJ])
            nc.vector.tensor_add(G4[:, :, 3, 0:NJ], G4[:, :, 3, 0:NJ], tr_b)

            # ---- transpose G into G_T (rows 32b+j), col a*NPC + ch*128+n --
            for a in range(3):
                gt_ps = ps1.tile([128, 128], F32, tag="gt")
                nc.tensor.transpose(gt_ps[:, :], G[:, a * 128:(a + 1) * 128],
                                    ident[:, :])
                nc.scalar.copy(G_T[:, a * NPC + ch * 128:
                                   a * NPC + ch * 128 + 128], gt_ps[:, :])

    # ================= phase 2: vertex tiles ================================
    wpool = ctx.enter_context(tc.tile_pool(name="wstream", bufs=4))
    mpool = ctx.enter_context(tc.tile_pool(name="mtiles", bufs=5))
    vout = ctx.enter_context(tc.tile_pool(name="vout", bufs=4))
    vppool = ctx.enter_context(tc.tile_pool(name="vpsb", bufs=2))
    ps_vp_pool = ctx.enter_context(
        tc.tile_pool(name="psvp", bufs=1, space="PSUM"))
    ps_t_pool = ctx.enter_context(
        tc.tile_pool(name="pst", bufs=2, space="PSUM"))
    ps_jt_pool = ctx.enter_context(
        tc.tile_pool(name="psjt", bufs=1, space="PSUM"))

    jt_ps = [ps_jt_pool.tile([NKP, 384], F32, tag="jt%d" % h,
                             name="jt_ps%d" % h)
             for h in range(2)]

    for vt in range(NT):
        cs = slice(vt * 384, (vt + 1) * 384)
        wt_hi = wpool.tile([128, 384], MMDT, tag="whi")
        nc.sync.dma_start(wt_hi[:, :], wt[0:128, cs])
        wt_lo = wpool.tile([128, 384], MMDT, tag="wlo")
        nc.sync.dma_start(wt_lo[0:90, :], wt[128:NF, cs])

        # ---- v_posed tile:  [128v, (c,n)] ----
        ps_vp = ps_vp_pool.tile([128, 3 * NPC], F32, tag="vp", name="ps_vp")
        whi3 = wt_hi[:, :].rearrange("p (v c) -> p c v", c=3)
        wlo3 = wt_lo[0:90, :].rearrange("p (v c) -> p c v", c=3)
        for c in range(3):
            nc.tensor.matmul(ps_vp[:, c * NPC:(c + 1) * NPC],
                             _mmdt(whi3[:, c, :]), _mmdt(F_hi[:, :]),
                             start=True, stop=False)
            nc.tensor.matmul(ps_vp[:, c * NPC:(c + 1) * NPC],
                             _mmdt(wlo3[:, c, :]), _mmdt(F_lo[0:90, :]),
                             start=False, stop=True)
        vp_sb = vppool.tile([128, 3 * NPC], F32, tag="vpsb")
        nc.scalar.copy(vp_sb[:, :], ps_vp[:, :])

        # ---- T slots + stage B ----
        m = []
        ps_t3 = None
        for b in range(4):
            ps_t = ps_t_pool.tile([128, 3 * NPC], F32, tag="pst", name="ps_t")
            # two mms (N=512 bank0, N=256 bank1) instead of three N=256:
            # fewer LDWEIGHTS and better fp32r streaming rate.
            for (o, w) in ((0, 512), (512, 256)):
                nc.tensor.matmul(
                    ps_t[:, o:o + w],
                    _mmdt(w4_sb[32 * b:32 * b + NJ,
                                vt * 128:(vt + 1) * 128]),
                    _mmdt(G_T[32 * b:32 * b + NJ, o:o + w]),
                    start=True, stop=True, tile_position=(32 * b, 0))
            if b < 3:
                mb = mpool.tile([128, 3 * NPC], F32, tag="m%d" % b)
                m3 = mb[:, :].rearrange("p (a n) -> p a n", n=NPC)
                t3 = ps_t[:, :].rearrange("p (a n) -> p a n", n=NPC)
                vp_b = vp_sb[:, b * NPC:(b + 1) * NPC].unsqueeze(1) \
                    .broadcast_to([128, 3, NPC])
                nc.vector.tensor_mul(m3, t3, vp_b)
                m.append(mb)
            else:
                ps_t3 = ps_t

        # adds on GpSimd: fp32 tensor_tensor on DVE is 1x/single-port, so
        # GpSimd runs fully in parallel (no shared-port contention).
        s1 = mpool.tile([128, 3 * NPC], F32, tag="s1")
        nc.gpsimd.tensor_tensor(s1[:, :], m[0][:, :], m[1][:, :],
                                mybir.AluOpType.add)
        s2 = mpool.tile([128, 3 * NPC], F32, tag="s2")
        nc.gpsimd.tensor_tensor(s2[:, :], s1[:, :], m[2][:, :],
                                mybir.AluOpType.add)
        verts_sb = vout.tile([128, 3 * NPC], F32, tag="verts")
        nc.vector.tensor_add(verts_sb[:, :], s2[:, :], ps_t3[:, :])

        # ---- joints accumulation (rounded copy keeps verts_sb full fp32) --
        if USE_F32R:
            verts_r = vout.tile([128, 3 * NPC], MMDT, tag="verts_r")
            nc.scalar.copy(verts_r[:, :], verts_sb[:, :])
            jrhs = verts_r
        else:
            jrhs = verts_sb
        for h in range(2):
            nc.tensor.matmul(jt_ps[h][0:NKP, :],
                             _mmdt(jreg_sb[:, vt * NKP:(vt + 1) * NKP]),
                             _mmdt(jrhs[:, h * 384:(h + 1) * 384]),
                             start=(vt == 0), stop=(vt == NT - 1))

        nc.sync.dma_start(verts_o[vt * 128:(vt + 1) * 128, :], verts_sb[:, :])

    joints_sb = vout.tile([NKP, 3 * NPC], F32, tag="joints")
    for h in range(2):
        nc.scalar.copy(joints_sb[0:NKP, h * 384:(h + 1) * 384],
                       jt_ps[h][0:NKP, :])
    nc.sync.dma_start(joints_o[:, :], joints_sb[0:NK, :])


# ======================= host side ========================================

def _prep_shared(shapedirs, v_template, J_regressor, posedirs, weights,
                 joint_regressor):
    wt = np.zeros((NF, VC_PAD), np.float32)
    wt[0, :V * 3] = v_template.ravel()
    wt[1:1 + NB, :V * 3] = shapedirs.transpose(2, 0, 1).reshape(NB, -1)
    wt[1 + NB:, :V * 3] = posedirs.transpose(2, 0, 1).reshape(NP, -1)

    w4 = np.zeros((128, V_PAD), np.float32)
    for u in range(4):
        w4[32 * u:32 * u + NJ, :V] = weights.T

    jr = np.zeros((V_PAD, NKP), np.float32)
    jr[:V, :NK] = joint_regressor
    jregp = np.ascontiguousarray(
        jr.reshape(NT, 128, NKP).transpose(1, 0, 2).reshape(128, NT * NKP))

    # JD[f, c*24+j] = sum_v [v_template|shapedirs][v,c,f] * J_regressor[j,v]
    jd = np.zeros((11, 3 * NJ), np.float32)
    for c in range(3):
        W_c = np.concatenate([v_template[:, c:c + 1], shapedirs[:, c, :]],
                             axis=1)                      # [V, 11]
        jd[:, c * NJ:(c + 1) * NJ] = (W_c.T @ J_regressor.T).astype(
            np.float32)

    return wt, w4, jregp, jd


def make_in_maps(inputs):
    """inputs: dict of full-size numpy arrays keyed as in setup_inputs()."""
    beta = np.asarray(inputs["beta"], np.float32)
    theta = np.asarray(inputs["theta"], np.float32)
    trans = np.asarray(inputs["trans"], np.float32)
    wt, w4, jregp, jd = _prep_shared(
        np.asarray(inputs["shapedirs"], np.float32),
        np.asarray(inputs["v_template"], np.float32),
        np.asarray(inputs["J_regressor"], np.float32),
        np.asarray(inputs["posedirs"], np.float32),
        np.asarray(inputs["weights"], np.float32),
        np.asarray(inputs["joint_regressor"], np.float32))

    in_maps = []
    for r in range(NCORES):
        ns = slice(r * NPC, (r + 1) * NPC)
        in_maps.append({
            "wt": wt, "w4": w4, "jregp": jregp, "jd": jd,
            "betaT": np.ascontiguousarray(
                np.concatenate([np.ones((1, NPC), np.float32),
                                beta[ns].T], axis=0)),
            "theta": np.ascontiguousarray(theta[ns].reshape(NPC, NJ * 3)),
            "trans": np.ascontiguousarray(trans[ns]),
        })
    return in_maps


def unshard(results):
    verts = np.empty((N_FULL, V, 3), np.float32)
    joints = np.empty((N_FULL, NK, 3), np.float32)
    for r, res in enumerate(results):
        ns = slice(r * NPC, (r + 1) * NPC)
        vo = res["verts_o"].reshape(V_PAD, 3, NPC)[:V]
        verts[ns] = vo.transpose(2, 0, 1)
        jo = res["joints_o"].reshape(NK, 3, NPC)
        joints[ns] = jo.transpose(2, 0, 1)
    return verts, joints


def get_nc():
    global _CACHED_NC
    if _CACHED_NC is None:
        _CACHED_NC = _build_nc()
    return _CACHED_NC


def kernel(**inputs):
    from concourse.bass_utils import run_bass_kernel_spmd
    nc = get_nc()
    in_maps = make_in_maps(inputs)
    res = run_bass_kernel_spmd(nc, in_maps, core_ids=list(range(NCORES)))
    return unshard(res.results)
